# revision 4
# baseline (speedup 1.0000x reference)
"""DyConv (dynamic convolution) Trainium2 kernel.

Problem: B=16, C=256, O=256, K=4 experts, 3x3 same-conv on 64x64, with
per-sample attention over experts + InstanceNorm2d(affine=False) input norm.

Strategy: data-parallel over batch across 8 cores (2 samples/core).
Each core:
  - loads its 2 samples of x (fp32) + the full expert weight bank (bf16,
    pre-transposed on host to [K, ctile, 128c, 9*256o]).
  - instance-norm stats via bn_stats/bn_aggr (DVE), normalization fused
    into one ACT pass that also casts to bf16 into a zero-padded 66x66
    spatial layout (so conv taps are plain AP offsets).
  - attention MLP in fp32 on PE (tiny matmuls), softmax via Exp on ACT +
    partition-sum matmul + DRAM round-trip broadcast.
  - per-sample weight aggregation on DVE as scalar_tensor_tensor FMA chain.
  - conv: for each (sample, otile, quarter of rows), accumulate
    2 ctile x 9 tap bf16 matmuls into PSUM; drain on ACT fused with the
    per-sample aggregated bias; DMA to DRAM.
"""

import sys

sys.path.insert(0, "/opt/trn_rl_repo")

import numpy as np
import ml_dtypes

import concourse.bacc as bacc
import concourse.tile as tile
from concourse import mybir
from concourse.bass_utils import run_bass_kernel_spmd

F32 = mybir.dt.float32
BF16 = mybir.dt.bfloat16
AF = mybir.ActivationFunctionType
ALU = mybir.AluOpType

N_CORES = 8
S = 2          # samples per core
C = 256        # in channels
O = 256        # out channels
K = 4          # experts
H = W = 64
HP = WP = 66   # padded spatial
NCT = 2        # C tiles of 128
NOT = 2        # O tiles of 128
EPS = 1e-5
TAPS = [(dy, dx) for dy in (-1, 0, 1) for dx in (-1, 0, 1)]


def build_program(trace_friendly: bool = False):
    nc = bacc.Bacc("TRN2", target_bir_lowering=False, debug=False,
                   num_devices=N_CORES)

    x_d = nc.dram_tensor("x", [S, C, H, W], F32, kind="ExternalInput")
    wt_d = nc.dram_tensor("wt", [K, NCT, 128, 9 * O], BF16, kind="ExternalInput")
    bias_d = nc.dram_tensor("bias", [K, O], F32, kind="ExternalInput")
    fc1wT_d = nc.dram_tensor("fc1wT", [NCT, 128, K], F32, kind="ExternalInput")
    fc1b_d = nc.dram_tensor("fc1b", [K, 1], F32, kind="ExternalInput")
    fc2wT_d = nc.dram_tensor("fc2wT", [K, K], F32, kind="ExternalInput")
    fc2b_d = nc.dram_tensor("fc2b", [K, 1], F32, kind="ExternalInput")
    out_d = nc.dram_tensor("out", [S, O, H, W], F32, kind="ExternalOutput")

    xap = x_d.ap()
    outap = out_d.ap()

    with tile.TileContext(nc) as tc:
        with (
            tc.tile_pool(name="singles", bufs=1) as singles,
            tc.tile_pool(name="xraw", bufs=3) as xraw_pool,
            tc.tile_pool(name="xn", bufs=4) as xn_pool,
            tc.tile_pool(name="acc", bufs=2) as acc_pool,
            tc.tile_pool(name="aggw", bufs=4) as aggw_pool,
            tc.tile_pool(name="stats", bufs=4) as stats_pool,
            tc.tile_pool(name="small", bufs=2) as small_pool,
            tc.tile_pool(name="outs", bufs=3) as out_pool,
            tc.tile_pool(name="cpsum", bufs=3, space="PSUM") as cpsum_pool,
            tc.tile_pool(name="apsum", bufs=2, space="PSUM") as apsum_pool,
            tc.tile_pool(name="dram", bufs=2, space="DRAM") as dram_pool,
        ):
            # ---- constants / small weights ----
            eps_sb = singles.tile([128, 1], F32, tag="eps")
            nc.vector.memset(eps_sb[:], EPS)
            ones_sb = singles.tile([K, 1], F32, tag="ones")
            nc.vector.memset(ones_sb[:], 1.0)

            fc1wT_sb = []
            for ci in range(NCT):
                t = singles.tile([128, K], F32, tag=f"fc1wT{ci}")
                nc.sync.dma_start(out=t[:], in_=fc1wT_d.ap()[ci])
                fc1wT_sb.append(t)
            fc2wT_sb = singles.tile([K, K], F32, tag="fc2wT")
            nc.sync.dma_start(out=fc2wT_sb[:], in_=fc2wT_d.ap())
            fc1b_sb = singles.tile([K, 1], F32, tag="fc1b")
            nc.sync.dma_start(out=fc1b_sb[:], in_=fc1b_d.ap())
            fc2b_sb = singles.tile([K, 1], F32, tag="fc2b")
            nc.sync.dma_start(out=fc2b_sb[:], in_=fc2b_d.ap())
            bias_sb = singles.tile([K, O], F32, tag="biasK")
            nc.sync.dma_start(out=bias_sb[:], in_=bias_d.ap())

            # ---- big loads: x (both samples), then expert weight bank ----
            x_raw = [[None] * NCT for _ in range(S)]
            for s in range(S):
                for ci in range(NCT):
                    t = xraw_pool.tile([128, H, W], F32, tag="xraw")
                    nc.sync.dma_start(
                        out=t[:], in_=xap[s, ci * 128:(ci + 1) * 128, :, :])
                    x_raw[s][ci] = t

            wt_sb = [[None] * NCT for _ in range(K)]
            for ci in range(NCT):
                for k in range(K):
                    t = singles.tile([128, 9 * O], BF16, tag=f"wt{k}_{ci}")
                    nc.sync.dma_start(out=t[:], in_=wt_d.ap()[k, ci])
                    wt_sb[k][ci] = t

            # ---- per-sample: stats + attention ----
            mv = [[None] * NCT for _ in range(S)]
            attn_bc = [None] * S   # [128, K] fp32, normalized attn broadcast
            aggb_sb = [[None] * NOT for _ in range(S)]

            for s in range(S):
                # instance-norm stats (also yields the attention GAP input)
                for ci in range(NCT):
                    st = stats_pool.tile([128, 8, 6], F32, tag="bnstats")
                    for j in range(8):
                        nc.vector.bn_stats(
                            out=st[:, j, :],
                            in_=x_raw[s][ci][:, 8 * j:8 * (j + 1), :]
                            .rearrange("p a b -> p (a b)"))
                    m = stats_pool.tile([128, 2], F32, tag="mv")
                    nc.vector.bn_aggr(out=m[:], in_=st[:])
                    mv[s][ci] = m

                # attention MLP (fp32, tiny)
                ph = apsum_pool.tile([K, 1], F32, tag="aps")
                for ci in range(NCT):
                    nc.tensor.matmul(ph[:], fc1wT_sb[ci][:], mv[s][ci][:, 0:1],
                                     start=(ci == 0), stop=(ci == NCT - 1))
                h_sb = small_pool.tile([K, 1], F32, tag="h")
                nc.scalar.activation(h_sb[:], ph[:], AF.Relu, bias=fc1b_sb[:])
                pl = apsum_pool.tile([K, 1], F32, tag="aps")
                nc.tensor.matmul(pl[:], fc2wT_sb[:], h_sb[:],
                                 start=True, stop=True)
                exp_t = small_pool.tile([K, 1], F32, tag="expt")
                nc.scalar.activation(exp_t[:], pl[:], AF.Exp, bias=fc2b_sb[:])
                psu = apsum_pool.tile([1, 1], F32, tag="aps")
                nc.tensor.matmul(psu[:], ones_sb[:], exp_t[:],
                                 start=True, stop=True)
                s_sb = small_pool.tile([1, 1], F32, tag="ssb")
                nc.vector.tensor_copy(s_sb[:], psu[:])

                # DRAM round trip to broadcast exp/sum across partitions
                rt = dram_pool.tile([1, 8], F32, tag="rt")
                nc.gpsimd.dma_start(out=rt[0:1, 0:K], in_=exp_t[:])
                nc.gpsimd.dma_start(out=rt[0:1, K:K + 1], in_=s_sb[:])
                exp_bc = small_pool.tile([128, K], F32, tag="expbc")
                nc.gpsimd.dma_start(out=exp_bc[:],
                                    in_=rt[0:1, 0:K].to_broadcast([128, K]))
                s_bc = small_pool.tile([128, 1], F32, tag="sbc")
                nc.gpsimd.dma_start(out=s_bc[:],
                                    in_=rt[0:1, K:K + 1].to_broadcast([128, 1]))
                r_bc = small_pool.tile([128, 1], F32, tag="rbc")
                nc.vector.reciprocal(out=r_bc[:], in_=s_bc[:])
                abc = small_pool.tile([128, K], F32, tag="attnbc")
                nc.vector.tensor_scalar(abc[:], exp_bc[:], r_bc[:, 0:1], None,
                                        ALU.mult)
                attn_bc[s] = abc
                attn_t = small_pool.tile([K, 1], F32, tag="attnt")
                nc.vector.tensor_mul(attn_t[:], exp_t[:], r_bc[0:K, 0:1])

                # aggregated bias agg_b[o] = sum_k attn[k] bias[k, o]
                for oi in range(NOT):
                    pab = apsum_pool.tile([128, 1], F32, tag="aps")
                    nc.tensor.matmul(pab[:],
                                     bias_sb[:, oi * 128:(oi + 1) * 128],
                                     attn_t[:], start=True, stop=True)
                    ab = singles.tile([128, 1], F32, tag=f"aggb{s}_{oi}")
                    nc.vector.tensor_copy(ab[:], pab[:])
                    aggb_sb[s][oi] = ab

            # ---- per-sample: normalize (into padded bf16) + weight agg ----
            xn = [[None] * NCT for _ in range(S)]
            aggw = [[None] * NCT for _ in range(S)]
            for s in range(S):
                for ci in range(NCT):
                    # rsqrt(var + eps), -mu * rsqrt
                    sd = stats_pool.tile([128, 1], F32, tag="sd")
                    nc.scalar.activation(sd[:], mv[s][ci][:, 1:2], AF.Sqrt,
                                         bias=eps_sb[:])
                    rs = stats_pool.tile([128, 1], F32, tag="rs")
                    nc.vector.reciprocal(out=rs[:], in_=sd[:])
                    nmrs = stats_pool.tile([128, 1], F32, tag="nmrs")
                    nc.vector.tensor_scalar(nmrs[:], mv[s][ci][:, 0:1],
                                            rs[:, 0:1], -1.0, ALU.mult,
                                            ALU.mult)

                    xt = xn_pool.tile([128, HP, WP], BF16, tag="xn")
                    nc.gpsimd.memset(xt[:], 0.0)
                    nc.scalar.activation(xt[:, 1:1 + H, 1:1 + W],
                                         x_raw[s][ci][:], AF.Identity,
                                         bias=nmrs[:, 0:1], scale=rs[:, 0:1])
                    xn[s][ci] = xt

                    # aggregate expert weights: fp32 FMA chain, bf16 out
                    ac = acc_pool.tile([128, 9 * O], F32, tag="acc")
                    nc.vector.tensor_scalar(ac[:], wt_sb[0][ci][:],
                                            attn_bc[s][:, 0:1], None, ALU.mult)
                    for k in (1, 2):
                        nc.vector.scalar_tensor_tensor(
                            ac[:], wt_sb[k][ci][:], attn_bc[s][:, k:k + 1],
                            ac[:], ALU.mult, ALU.add)
                    aw = aggw_pool.tile([128, 9, O], BF16, tag="aggw")
                    nc.vector.scalar_tensor_tensor(
                        aw[:].rearrange("p a b -> p (a b)"), wt_sb[3][ci][:],
                        attn_bc[s][:, 3:4], ac[:], ALU.mult, ALU.add)
                    aggw[s][ci] = aw

            # ---- conv + bias + store ----
            for s in range(S):
                for oi in range(NOT):
                    for q in range(4):  # quarters of 16 output rows
                        ps = cpsum_pool.tile([128, 1024], F32, tag="cps")
                        for ci in range(NCT):
                            for t, (dy, dx) in enumerate(TAPS):
                                lhsT = aggw[s][ci][:, t, oi * 128:(oi + 1) * 128]
                                first = (ci == 0 and t == 0)
                                last = (ci == NCT - 1 and t == len(TAPS) - 1)
                                for blk in range(2):
                                    y0 = q * 16 + blk * 8
                                    rhs = xn[s][ci][:, y0 + 1 + dy:y0 + 9 + dy,
                                                    1 + dx:1 + dx + W]
                                    nc.tensor.matmul(
                                        ps[:, blk * 512:(blk + 1) * 512],
                                        lhsT, rhs, start=first, stop=last)
                        ot = out_pool.tile([128, 1024], F32, tag="ot")
                        nc.scalar.activation(ot[:], ps[:], AF.Identity,
                                             bias=aggb_sb[s][oi][:, 0:1])
                        nc.gpsimd.dma_start(
                            out=outap[s, oi * 128:(oi + 1) * 128,
                                      q * 16:(q + 1) * 16, :],
                            in_=ot[:])

    nc.compile()
    return nc


_CACHED = {}


def _get_program():
    if "nc" not in _CACHED:
        _CACHED["nc"] = build_program()
    return _CACHED["nc"]


def _prep_shared(weight, bias, fc1_w, fc1_b, fc2_w, fc2_b):
    # weight [K, O, C, 3, 3] -> [K, C, 3*3, O] -> [K, NCT, 128, 9*O], bf16
    wt = np.ascontiguousarray(weight.transpose(0, 2, 3, 4, 1)).reshape(
        K, NCT, 128, 9 * O).astype(ml_dtypes.bfloat16)
    fc1wT = np.ascontiguousarray(fc1_w.T).reshape(NCT, 128, K).astype(np.float32)
    fc2wT = np.ascontiguousarray(fc2_w.T).astype(np.float32)
    return {
        "wt": wt,
        "bias": bias.astype(np.float32),
        "fc1wT": fc1wT,
        "fc1b": fc1_b.reshape(K, 1).astype(np.float32),
        "fc2wT": fc2wT,
        "fc2b": fc2_b.reshape(K, 1).astype(np.float32),
    }


def run(x, weight, bias, fc1_w, fc1_b, fc2_w, fc2_b, trace=False,
        trace_kwargs=None):
    nc = _get_program()
    shared = _prep_shared(weight, bias, fc1_w, fc1_b, fc2_w, fc2_b)
    x = np.asarray(x, dtype=np.float32)
    in_maps = []
    for i in range(N_CORES):
        m = dict(shared)
        m["x"] = np.ascontiguousarray(x[i * S:(i + 1) * S])
        in_maps.append(m)
    res = run_bass_kernel_spmd(nc, in_maps, core_ids=list(range(N_CORES)),
                               trace=trace, **(trace_kwargs or {}))
    out = np.concatenate([res.results[i]["out"] for i in range(N_CORES)],
                         axis=0)
    return out, res


def kernel(x, weight, bias, fc1_w, fc1_b, fc2_w, fc2_b):
    out, _ = run(x, weight, bias, fc1_w, fc1_b, fc2_w, fc2_b)
    return out
